# revision 5
# baseline (speedup 1.0000x reference)
"""Trainium2 Bass kernel for CBERT-linear ragged candidate scoring + CE loss.

Computation (reference semantics):
    mask   = sense_ids >= 0                         # [B, K]
    w, b   = weight[ids], bias[ids]                 # ragged gather
    logits = einsum('bkd,bd->bk', w, reps) + b      # masked to -1e30
    loss   = mean_b( logsumexp(logits_b) - logits_b[target_b] )
    correct= argmax(logits, -1) == target

Sharding: the [N, 768] weight/bias table is replicated to all 8 cores; the
batch is split 64 samples per core (data parallel).  Each core:
  1. indirect-DMA gathers its 2048 candidate rows (weight||bias packed into
     776-float rows) from the replicated table in HBM — 16 gathers of
     [128 rows x 776 f32],
  2. one fused DVE scalar_tensor_tensor per gather computes the per-row dot
     product against a resident reps tile (accum_out = row-sum of products),
  3. a small masked log-softmax / argmax tail produces nll[64] and
     correct[64].
Host glue only reorders inputs, averages the 8x64 nll values and
concatenates the correct flags.

Row-slot layout per core: slot (p, j), p in [0,128), j in [0,16) holds
sample b = p % 64, candidate k = j + 16*(p // 64).  reps tile row p is
therefore rep[p % 64] for every j, so one resident [128, 776] tile serves
all 16 gathers.
"""

import numpy as np

import concourse.bacc as bacc
import concourse.bass as bass
import concourse.mybir as mybir
import concourse.tile as tile
from concourse.bass_utils import run_bass_kernel_spmd

F32 = mybir.dt.float32
I32 = mybir.dt.int32
ALU = mybir.AluOpType
ACTFN = mybir.ActivationFunctionType

N_CORES = 8
B = 512            # total batch
K = 32             # max candidates per sample
D = 768            # feature dim
BC = B // N_CORES  # samples per core (64)
NJ = K * BC // 128  # gather instructions per core (16)
DROW = 776         # packed table row: 768 weight + 1 bias + 7 zero pad
NEG_INF = -1e30
BIGF = 65536.0     # argmin sentinel; k + (k - BIGF) + BIGF is exact in f32

# aux tensor column layout (per-core [64, AUXW] f32)
A_REP = 0                 # [0,776): [reps | 1.0 | 0*7]
A_MASK = DROW             # [776,808): 0 where valid else -1e30
A_IOTA = A_MASK + K       # [808,840): k - BIGF
A_HOT = A_IOTA + K        # [840,872): one-hot(target)
A_TGT = A_HOT + K         # [872,873): target as f32
AUXW = A_TGT + 1


def build_nc(n_rows: int, g_bufs: int = 16, reps_body: int = 1):
    nc = bacc.Bacc("TRN2", target_bir_lowering=False, debug=False,
                   num_devices=N_CORES)
    wb = nc.dram_tensor("wb", [n_rows, DROW], F32, kind="ExternalInput")
    idx = nc.dram_tensor("idx", [128, NJ], I32, kind="ExternalInput")
    aux = nc.dram_tensor("aux", [BC, AUXW], F32, kind="ExternalInput")
    out = nc.dram_tensor("out", [BC, 2], F32, kind="ExternalOutput")

    with tile.TileContext(nc) as tc:
        with tc.tile_pool(name="gp", bufs=g_bufs) as gp, \
             tc.tile_pool(name="sp", bufs=1) as sp:
            idx_sb = sp.tile([128, NJ], I32)
            r_sb = sp.tile([128, DROW], F32)
            aux_sb = sp.tile([BC, AUXW], F32)

            nc.sync.dma_start(out=idx_sb[:], in_=idx[:])
            nc.sync.dma_start(out=aux_sb[:], in_=aux[:])
            # reps tile: rep row p%64 on every partition
            nc.sync.dma_start(out=r_sb[0:BC, :], in_=aux[:, A_REP:DROW])
            nc.sync.dma_start(out=r_sb[BC:128, :], in_=aux[:, A_REP:DROW])

            for _rep in range(reps_body):
                _emit_body(nc, gp, sp, wb, out, idx_sb, r_sb, aux_sb)
    nc.compile()
    return nc


def _emit_body(nc, gp, sp, wb, out, idx_sb, r_sb, aux_sb):
            lcols = sp.tile([128, NJ], F32)
            for j in range(NJ):
                g = gp.tile([128, DROW], F32, tag="g")
                nc.gpsimd.indirect_dma_start(
                    out=g[:], out_offset=None,
                    in_=wb[:],
                    in_offset=bass.IndirectOffsetOnAxis(ap=idx_sb[:, j:j + 1],
                                                        axis=0),
                )
                # g = g * r ; lcols[:, j] = sum_d g[:, d]  (the row dot)
                nc.vector.scalar_tensor_tensor(
                    out=g[:], in0=g[:], scalar=1.0, in1=r_sb[:],
                    op0=ALU.bypass, op1=ALU.mult,
                    accum_out=lcols[:, j:j + 1],
                )

            # ---- per-sample tail on 64 partitions ----
            # sample b's k>=16 logits live on partition b+64; DMA them down
            # (engine ops can't shift partitions, and 2-input ops need equal
            # base partitions)
            hi = sp.tile([BC, NJ], F32)
            nc.sync.dma_start(out=hi[:], in_=lcols[BC:128, :])
            L = sp.tile([BC, K], F32)
            nc.vector.tensor_tensor(out=L[:, 0:NJ], in0=lcols[0:BC, :],
                                    in1=aux_sb[:, A_MASK:A_MASK + NJ],
                                    op=ALU.add)
            nc.vector.tensor_tensor(out=L[:, NJ:K], in0=hi[:],
                                    in1=aux_sb[:, A_MASK + NJ:A_MASK + K],
                                    op=ALU.add)

            mx = sp.tile([BC, 1], F32)
            nmx = sp.tile([BC, 1], F32)
            nc.vector.reduce_max(out=mx[:], in_=L[:], axis=mybir.AxisListType.X)
            nc.vector.tensor_scalar(out=nmx[:], in0=mx[:], scalar1=-1.0,
                                    scalar2=None, op0=ALU.mult)

            e = sp.tile([BC, K], F32)
            se = sp.tile([BC, 1], F32)
            lse = sp.tile([BC, 1], F32)
            nc.scalar.activation(out=e[:], in_=L[:], func=ACTFN.Exp,
                                 bias=nmx[:, 0:1], scale=1.0,
                                 accum_out=se[:])
            nc.scalar.activation(out=lse[:], in_=se[:], func=ACTFN.Ln)

            tp = sp.tile([BC, K], F32)
            tgt = sp.tile([BC, 1], F32)
            nc.vector.tensor_tensor(out=tp[:], in0=L[:],
                                    in1=aux_sb[:, A_HOT:A_HOT + K],
                                    op=ALU.mult)
            nc.vector.reduce_sum(out=tgt[:], in_=tp[:],
                                 axis=mybir.AxisListType.X)

            out_sb = sp.tile([BC, 2], F32)
            s1 = sp.tile([BC, 1], F32)
            nc.vector.tensor_tensor(out=s1[:], in0=mx[:], in1=lse[:],
                                    op=ALU.add)
            nc.vector.tensor_tensor(out=out_sb[:, 0:1], in0=s1[:], in1=tgt[:],
                                    op=ALU.subtract)

            eq = sp.tile([BC, K], F32)
            c2 = sp.tile([BC, K], F32)
            minidx = sp.tile([BC, 1], F32)
            nc.vector.tensor_tensor(out=eq[:], in0=L[:],
                                    in1=mx[:, 0:1].to_broadcast([BC, K]),
                                    op=ALU.is_equal)
            nc.vector.tensor_tensor(out=c2[:], in0=eq[:],
                                    in1=aux_sb[:, A_IOTA:A_IOTA + K],
                                    op=ALU.mult)
            nc.vector.tensor_scalar(out=c2[:], in0=c2[:], scalar1=BIGF,
                                    scalar2=None, op0=ALU.add)
            nc.vector.tensor_reduce(out=minidx[:], in_=c2[:],
                                    axis=mybir.AxisListType.X, op=ALU.min)
            nc.vector.tensor_tensor(out=out_sb[:, 1:2], in0=minidx[:],
                                    in1=aux_sb[:, A_TGT:A_TGT + 1],
                                    op=ALU.is_equal)

            nc.sync.dma_start(out=out[:], in_=out_sb[:])


def prep_inputs(reps, weight, bias, sense_ids, target_ids):
    """Host-side input marshalling -> (wb, per-core in_maps)."""
    reps = np.ascontiguousarray(np.asarray(reps, dtype=np.float32))
    weight = np.asarray(weight, dtype=np.float32)
    bias = np.asarray(bias, dtype=np.float32)
    ids = np.asarray(sense_ids).astype(np.int64)
    tgt = np.asarray(target_ids).astype(np.int64)
    n_rows = weight.shape[0]

    wb = np.zeros((n_rows, DROW), dtype=np.float32)
    wb[:, :D] = weight
    wb[:, D] = bias

    valid = ids >= 0
    idc = np.where(valid, ids, 0).astype(np.int32)

    kk = np.arange(K, dtype=np.float32)
    in_maps = []
    for c in range(N_CORES):
        s = slice(c * BC, (c + 1) * BC)
        ids_s = idc[s]                      # [64, 32]
        idx_host = np.empty((128, NJ), dtype=np.int32)
        idx_host[:BC] = ids_s[:, :NJ]
        idx_host[BC:] = ids_s[:, NJ:]

        aux = np.zeros((BC, AUXW), dtype=np.float32)
        aux[:, :D] = reps[s]
        aux[:, D] = 1.0
        aux[:, A_MASK:A_MASK + K] = np.where(valid[s], 0.0, NEG_INF)
        aux[:, A_IOTA:A_IOTA + K] = kk - BIGF
        hot = np.zeros((BC, K), dtype=np.float32)
        hot[np.arange(BC), tgt[s]] = 1.0
        aux[:, A_HOT:A_HOT + K] = hot
        aux[:, A_TGT] = tgt[s].astype(np.float32)
        in_maps.append({"wb": wb, "idx": idx_host, "aux": aux})
    return in_maps


def combine_outputs(results, target_ids):
    nll = np.concatenate([r["out"][:, 0] for r in results])
    correct = np.concatenate([r["out"][:, 1] for r in results]) > 0.5
    loss = np.float32(nll.mean(dtype=np.float64))
    return loss, correct


_NC_CACHE = {}


def kernel(reps, weight, bias, sense_ids, target_ids):
    n_rows = np.asarray(weight).shape[0]
    if n_rows not in _NC_CACHE:
        _NC_CACHE[n_rows] = build_nc(n_rows)
    nc = _NC_CACHE[n_rows]
    in_maps = prep_inputs(reps, weight, bias, sense_ids, target_ids)
    res = run_bass_kernel_spmd(nc, in_maps, core_ids=list(range(N_CORES)))
    return combine_outputs(res.results, target_ids)
